# revision 45
# baseline (speedup 1.0000x reference)
"""Masked (expander) linear layer on 8 Trainium2 NeuronCores.

Computes out = x @ (W * M)^T for
  x: [16384, 2048] f32, W: [2048, 2048] f32, M: [2048, 2048] int32 (0/1)

Sharding: pure data-parallel over rows of x. Each of the 8 cores gets 2048
rows of x plus a replicated (transposed) copy of W and M, computes its
[2048, 2048] output shard entirely locally (mask-multiply on DVE, matmul on
PE), and the host concatenates shards. No collectives.

Device-side design (v5, bf16):
 - All matmuls run in bf16 (1 PE cycle/row -- same peak rate as f32r --
   but LDWEIGHTS gets Fast-Weight-Load, ~100ns vs ~227ns for fp32, so
   weight loads hide behind the 512-cycle moving stream; measured MATMUL
   spacing sits at the 216ns streaming floor). PSUM accumulates f32 over
   the full K=2048; outputs stored f32. Measured rel err ~2.2e-3.
 - Transport is bf16/int8 (host pre-packs; bit-identical to an on-device
   cast since mask is 0/1). All module arithmetic (mask multiply, matmul)
   stays on device.
 - Every input DMA piece is stored partition-contiguous on the host
   ([P, piece_bytes] blocks), so each transfer is 128 descriptors of
   4-8KB: HWDGE issue cost drops ~4x and transfers run at line rate.
   (The v4 strided-panel layout made each DMA 512x 1KB descriptors;
   descriptor generation capped each ring at ~175 GB/s and the W stream
   starved the PE for ~50us mid-ramp.)
 - Deadline-ordered ring assignment: sync ring carries W pieces 0-3 then
   x quarters 2-3 then W pieces 4-15; scalar carries the 16 masks (then
   evacuation work); gpsimd carries x quarters 0-1 and then all of block
   1's x, which FIFO-throttles itself behind the ramp-critical loads.
 - W pieces DMA directly into the resident wm tiles; masks are resident;
   the DVE mask-multiply runs in place (one op per piece).
 - m-tiles run in blocks of 8 (nt-outer inside a block) so each W piece
   unlocks 32 matmuls (~7us) vs ~2us delivery. x is double-buffered
   across blocks. PSUM groups rotate over all 8 banks; each group closes
   after its 16-kt accumulation and is evacuated immediately (copies
   alternate ScalarE/VectorE into a 4-group staging tile; one store DMA
   per 4 groups, alternating sync/scalar rings).
 - A short warmup burst of junk matmuls (zeroed tile, PSUM bank 0) runs
   during the first DMA wait so the PE HAM clock-gate is already at full
   rate (2.4 GHz) when the real matmul stream begins.
"""

from contextlib import ExitStack

import numpy as np

import concourse.bacc as bacc
import concourse.bass as bass
import concourse.mybir as mybir
import concourse.tile as tile
from concourse.bass_utils import run_bass_kernel_spmd

N_CORES = 8
P = 128

FULL_N, FULL_OUT, FULL_IN = 16384, 2048, 2048


def build_nc(
    rows: int = FULL_N // N_CORES,
    in_dim: int = FULL_IN,
    out_dim: int = FULL_OUT,
    n_chunk: int = 512,
    m_block: int = 8,
    warmup_mms: int = 16,
    y_batch: int = 2,
):
    """Per-core Bass module: y[rows, out] = x @ (wt * m).

    DRAM layouts (all piece-contiguous, see _prep_host):
      wt [NT, 4, P, KQ*n_chunk] bf16   -- piece (nt, q) is [P, KQ*n_chunk]
      mk [NT, 4, P, KQ*n_chunk] int8
      x  [NB, 4, P, KQ*m_block*P] bf16 -- piece (b, q) is [P, KQ*mw]
      y  [rows, out_dim] f32 (row-major)
    """
    assert rows % P == 0 and in_dim % P == 0 and out_dim % n_chunk == 0
    KT = in_dim // P
    MT = rows // P
    NT = out_dim // n_chunk
    assert KT % 4 == 0 and MT % m_block == 0
    KQ = KT // 4
    NB = MT // m_block
    mw = m_block * P  # columns of x per block
    assert m_block % y_batch == 0

    bf16 = mybir.dt.bfloat16

    nc = bacc.Bacc("TRN2", target_bir_lowering=False, debug=False)
    x = nc.dram_tensor("x", [NB, 4, P, KQ * mw], bf16, kind="ExternalInput")
    wt = nc.dram_tensor("wt", [NT, 4, P, KQ * n_chunk], bf16, kind="ExternalInput")
    mk = nc.dram_tensor("mk", [NT, 4, P, KQ * n_chunk], mybir.dt.int8, kind="ExternalInput")
    y = nc.dram_tensor("y", [rows, out_dim], mybir.dt.float32, kind="ExternalOutput")

    with ExitStack() as ctx:
        tc = ctx.enter_context(tile.TileContext(nc))
        wm_pool = ctx.enter_context(tc.tile_pool(name="wm", bufs=1))
        mk_pool = ctx.enter_context(tc.tile_pool(name="mk", bufs=1))
        xt_pool = ctx.enter_context(tc.tile_pool(name="xt", bufs=1))
        yo_pool = ctx.enter_context(tc.tile_pool(name="yo", bufs=3))
        wu_pool = ctx.enter_context(tc.tile_pool(name="wu", bufs=1))
        pm_pool = ctx.enter_context(tc.tile_pool(name="pm", bufs=1, space="PSUM"))

        # Resident masked weight: wm_t[nt][q] of shape [P, KQ, n_chunk]
        wm_t = [
            [
                wm_pool.tile([P, KQ, n_chunk], bf16, tag=f"wm{nt}_{q}", name=f"wm{nt}_{q}")
                for q in range(4)
            ]
            for nt in range(NT)
        ]
        # Resident masks
        mk_t = [
            [
                mk_pool.tile(
                    [P, KQ, n_chunk], mybir.dt.int8, tag=f"mk{nt}_{q}", name=f"mk{nt}_{q}"
                )
                for q in range(4)
            ]
            for nt in range(NT)
        ]
        # x tiles: double-buffered per block parity: [set][q] -> [P, KQ, mw]
        xt_t = [
            [
                xt_pool.tile([P, KQ, mw], bf16, tag=f"xt{s}_{q}", name=f"xt{s}_{q}")
                for q in range(4)
            ]
            for s in range(2)
        ]

        # ---- PE warmup: junk matmuls into PSUM bank 0 ----
        # Values are irrelevant: the first real accumulation group on bank 0
        # opens with start=True, which clears has_written and overwrites.
        # Keeps the PE busy from the end of the engine-barrier preamble
        # (~6.5us) so HAM is at full clock when the real stream begins.
        if warmup_mms:
            wutile = wu_pool.tile([P, n_chunk], bf16, tag="wu", name="wu")
            nc.vector.memzero(wutile[:])
            pmw = pm_pool.tile([P, n_chunk], mybir.dt.float32, tag="pm0", name="pm_warm")
            for i in range(warmup_mms):
                nc.tensor.matmul(
                    pmw[:],
                    wutile[:, :P],
                    wutile[:],
                    start=(i == 0),
                    stop=(i == warmup_mms - 1),
                )

        def load_w_piece(nt, q):
            nc.sync.dma_start(
                out=wm_t[nt][q][:, :, :].rearrange("p a b -> p (a b)"),
                in_=wt[nt, q, :, :],
            )

        def mul_piece(nt, q):
            # per-k-slice muls: finer completion granularity, so the first
            # matmuls of a piece unblock ~2us earlier than one big mul
            for k in range(KQ):
                nc.vector.tensor_mul(
                    wm_t[nt][q][:, k, :], wm_t[nt][q][:, k, :], mk_t[nt][q][:, k, :]
                )

        def load_x_piece(b, q, eng):
            eng.dma_start(
                out=xt_t[b % 2][q][:, :, :].rearrange("p a b -> p (a b)"),
                in_=x[b, q, :, :],
            )

        def load_x_mslice(b, q, lo, hi, eng):
            xv = x[b, q, :, :].rearrange("p (a b) -> p a b", a=KQ)
            eng.dma_start(
                out=xt_t[b % 2][q][:, :, lo:hi],
                in_=xv[:, :, lo:hi],
            )

        # ---- prep: deadline-ordered across the two HWDGE rings ----
        # (gpsimd/SWDGE is avoided entirely: ~15us first-transfer latency and
        # it serializes issues on its own completion semaphores)
        # The nt0 window (~first 40us) is bandwidth-crunched: both HWDGE
        # rings carry deadline-critical bytes, split so W for nt1/nt3 rides
        # scalar (which goes quiet after the nt0 masks) instead of queuing
        # behind nt0's x quarters on sync.
        def load_mask(nt, q):
            nc.scalar.dma_start(out=mk_t[nt][q][:, :, :].rearrange("p a b -> p (a b)"),
                                in_=mk[nt, q, :, :])

        def load_w_piece_eng(nt, q, eng):
            eng.dma_start(
                out=wm_t[nt][q][:, :, :].rearrange("p a b -> p (a b)"),
                in_=wt[nt, q, :, :],
            )

        def load_w_khalf(nt, q, h, eng):
            # k-half of a W piece: more parallel streams early (HBM needs
            # outstanding requests to saturate); the per-k-slice muls gate on
            # subtile deps so the first slices unlock as soon as half A lands
            kh = KQ // 2
            eng.dma_start(
                out=wm_t[nt][q][:, h * kh : (h + 1) * kh, :].rearrange(
                    "p a b -> p (a b)"
                ),
                in_=wt[nt, q, :, h * kh * n_chunk : (h + 1) * kh * n_chunk],
            )

        # CONSTRUCTION ORDER MATTERS: a consumer must be issued after its
        # producers (dependencies only point backward in program order), so
        # each mask/W piece is constructed before the mul that reads it.
        # Ring plan -- the first-MM prefix is split across BOTH HWDGE rings
        # so its pieces transfer in parallel (a single early stream only
        # reaches ~150 GB/s; HBM needs several outstanding streams):
        #   sync:   xq0 (2 halves), W(0,2..3), xq3, W nt1-3, block-1 x
        #   scalar: W(0,0), m(0,0), W(0,1), m(0,1..3), masks nt1-3, evac/y
        #   gpsimd: xq1, xq2 only (SWDGE startup ~15us; deadlines 19/26us)
        # Block-1 x is last on sync: same-ring FIFO is the only real
        # throttle that keeps it out of the ramp crunch.
        load_w_piece_eng(0, 0, nc.scalar)
        load_mask(0, 0)
        mul_piece(0, 0)
        # xq0 streams in m-quarters SPLIT ACROSS BOTH RINGS in consumption
        # order: sync carries quarters 0-1 from t=0 while scalar (after the
        # small W00/m00 prefix) carries quarters 2-3 in parallel -- the
        # first matmuls start several us earlier than a serial xq0 stream
        load_x_mslice(0, 0, 0, mw // 4, nc.sync)
        load_x_mslice(0, 0, mw // 4, mw // 2, nc.sync)
        load_x_mslice(0, 0, mw // 2, 3 * mw // 4, nc.scalar)
        load_w_piece_eng(0, 1, nc.scalar)
        load_mask(0, 1)
        mul_piece(0, 1)
        load_x_mslice(0, 0, 3 * mw // 4, mw, nc.scalar)
        for q in (2, 3):
            load_w_piece(0, q)
            load_mask(0, q)
            mul_piece(0, q)
        load_x_piece(0, 1, nc.gpsimd)
        load_x_piece(0, 2, nc.gpsimd)
        load_x_piece(0, 3, nc.sync)
        # nt1's first W pieces ride gpsimd behind xq2: its big-descriptor
        # queue holds a high packet share, landing them ~20us before the
        # sync ring (busy with xq3 + the rest of W) could
        load_w_piece_eng(1, 0, nc.gpsimd)
        load_w_piece_eng(1, 1, nc.gpsimd)
        for q in range(4):
            load_mask(1, q)
        load_w_piece(1, 2)
        load_w_piece(1, 3)
        for nt in (2, 3):
            for q in range(4):
                load_w_piece(nt, q)
        for nt in (2, 3):
            for q in range(4):
                load_mask(nt, q)
        if NB > 1:
            for q in range(4):
                load_x_piece(1, q, nc.sync)
        # NOTE: muls for nt>=1 are issued inside the main loop (interleaved
        # with nt-1's matmuls) so VectorE's strict-FIFO queue never parks a
        # late mask-multiply in front of PSUM evacuation copies.

        # ---- main: blocks of m_block m-tiles; nt-outer inside a block ----
        evac_i = 0
        for b in range(NB):
            xts = xt_t[b % 2]
            for nt in range(NT):
                # 8 rotating PSUM banks: group (nt, mb) lives on bank mb
                pms = {
                    mb: pm_pool.tile(
                        [P, n_chunk],
                        mybir.dt.float32,
                        tag=f"pm{(nt * m_block + mb) % 8}",
                        name=f"pm{(nt * m_block + mb) % 8}",
                    )
                    for mb in range(m_block)
                }
                if b == 0 and nt + 1 < NT:
                    # issue next nt's mask-multiplies now: they complete
                    # (~11us) well inside this nt's ~27us of matmuls, and
                    # VectorE's FIFO stays clear for evacuation copies
                    for q in range(4):
                        mul_piece(nt + 1, q)
                yo = None
                for q in range(4):
                    for mb in range(m_block):
                        for k in range(KQ):
                            kt = q * KQ + k
                            nc.tensor.matmul(
                                pms[mb][:],
                                xts[q][:, k, bass.ts(mb, P)],
                                wm_t[nt][q][:, k, :],
                                start=(kt == 0),
                                stop=(kt == KT - 1),
                            )
                        if q == 3:
                            # evacuate on close: copy into a batch-wide
                            # staging tile, one store DMA per batch.
                            # Block 0: copies are Scalar-only -- VectorE's
                            # strict FIFO still holds pending mask-muls, and a
                            # copy parked behind them would delay PSUM bank
                            # frees. Block 1+: alternate Scalar/VectorE.
                            yb = y_batch
                            j = mb % y_batch
                            if j == 0:
                                yo = yo_pool.tile(
                                    [P, yb, n_chunk], mybir.dt.float32, tag="yo"
                                )
                            if b == 0 or evac_i % 2 == 0:
                                nc.scalar.copy(yo[:, j, :], pms[mb][:])
                            else:
                                nc.vector.tensor_copy(yo[:, j, :], pms[mb][:])
                            evac_i += 1
                            if j == yb - 1:
                                mt0 = b * m_block + mb - (yb - 1)
                                dst = y[
                                    mt0 * P : (mt0 + yb) * P, bass.ts(nt, n_chunk)
                                ].rearrange("(j p) n -> p j n", p=P)
                                # all stores on scalar: HWDGE pipelines issues
                                # without waiting on completions (SWDGE does
                                # not), and sync stays clear for inputs
                                nc.scalar.dma_start(out=dst, in_=yo[:])

    nc.compile()
    return nc


def _prep_host(input_, weight, mask, n_chunk=512, m_block=8):
    import ml_dtypes

    in_dim, out_dim = weight.shape[1], weight.shape[0]
    nt = out_dim // n_chunk
    kt = in_dim // P
    kq = kt // 4
    rows = input_.shape[0] // N_CORES
    mw = m_block * P
    nb = rows // mw

    # W.T [IN, OUT] -> pieces [NT, 4, P, KQ*n_chunk]:
    # piece (t, q)[p, kl*n_chunk + n] = W.T[(4q+kl)*128 + p, t*n_chunk + n]
    wtT = weight.T.reshape(4, kq, P, nt, n_chunk)  # [q, kl, p, t, n]
    wtp = np.ascontiguousarray(wtT.transpose(3, 0, 2, 1, 4)).reshape(
        nt, 4, P, kq * n_chunk
    ).astype(ml_dtypes.bfloat16)
    mkT = mask.T.reshape(4, kq, P, nt, n_chunk)
    mkp = np.ascontiguousarray(mkT.transpose(3, 0, 2, 1, 4)).reshape(
        nt, 4, P, kq * n_chunk
    ).astype(np.int8)

    xbf = input_.astype(ml_dtypes.bfloat16)
    in_maps = []
    for c in range(N_CORES):
        xc = xbf[c * rows : (c + 1) * rows].T  # [IN, rows]
        # pieces [NB, 4, P, KQ*mw]: (b, q)[p, kl*mw + m] = xc[(4q+kl)*128+p, b*mw+m]
        xr = xc.reshape(4, kq, P, nb, mw)  # [q, kl, p, b, m]
        xp = np.ascontiguousarray(xr.transpose(3, 0, 2, 1, 4)).reshape(
            nb, 4, P, kq * mw
        )
        in_maps.append({"x": xp, "wt": wtp, "mk": mkp})
    return in_maps


_CACHE = {}


def _run(input_, weight, mask, trace=False, **build_kw):
    rows_total, in_dim = input_.shape
    out_dim = weight.shape[0]
    key = (rows_total, in_dim, out_dim, tuple(sorted(build_kw.items())))
    if key not in _CACHE:
        _CACHE[key] = build_nc(
            rows=rows_total // N_CORES, in_dim=in_dim, out_dim=out_dim, **build_kw
        )
    nc = _CACHE[key]
    in_maps = _prep_host(
        input_,
        weight,
        mask,
        build_kw.get("n_chunk", 512),
        build_kw.get("m_block", 8),
    )
    res = run_bass_kernel_spmd(nc, in_maps, core_ids=list(range(N_CORES)), trace=trace)
    out = np.concatenate([res.results[c]["y"] for c in range(N_CORES)], axis=0)
    return out, res


def kernel(input_, weight, mask):
    input_ = np.asarray(input_, dtype=np.float32)
    weight = np.asarray(weight, dtype=np.float32)
    mask = np.asarray(mask)
    out, _ = _run(input_, weight, mask, trace=False)
    return out


# revision 46
# speedup vs baseline: 1.0209x; 1.0209x over previous
"""Masked (expander) linear layer on 8 Trainium2 NeuronCores.

Computes out = x @ (W * M)^T for
  x: [16384, 2048] f32, W: [2048, 2048] f32, M: [2048, 2048] int32 (0/1)

Sharding: pure data-parallel over rows of x. Each of the 8 cores gets 2048
rows of x plus a replicated (transposed) copy of W and M, computes its
[2048, 2048] output shard entirely locally (mask-multiply on DVE, matmul on
PE), and the host concatenates shards. No collectives.

Device-side design (v5, bf16):
 - All matmuls run in bf16 (1 PE cycle/row -- same peak rate as f32r --
   but LDWEIGHTS gets Fast-Weight-Load, ~100ns vs ~227ns for fp32, so
   weight loads hide behind the 512-cycle moving stream; measured MATMUL
   spacing sits at the 216ns streaming floor). PSUM accumulates f32 over
   the full K=2048; outputs stored f32. Measured rel err ~2.2e-3.
 - Transport is bf16/int8 (host pre-packs; bit-identical to an on-device
   cast since mask is 0/1). All module arithmetic (mask multiply, matmul)
   stays on device.
 - Every input DMA piece is stored partition-contiguous on the host
   ([P, piece_bytes] blocks), so each transfer is 128 descriptors of
   4-8KB: HWDGE issue cost drops ~4x and transfers run at line rate.
   (The v4 strided-panel layout made each DMA 512x 1KB descriptors;
   descriptor generation capped each ring at ~175 GB/s and the W stream
   starved the PE for ~50us mid-ramp.)
 - Deadline-ordered ring assignment: sync ring carries W pieces 0-3 then
   x quarters 2-3 then W pieces 4-15; scalar carries the 16 masks (then
   evacuation work); gpsimd carries x quarters 0-1 and then all of block
   1's x, which FIFO-throttles itself behind the ramp-critical loads.
 - W pieces DMA directly into the resident wm tiles; masks are resident;
   the DVE mask-multiply runs in place (one op per piece).
 - m-tiles run in blocks of 8 (nt-outer inside a block) so each W piece
   unlocks 32 matmuls (~7us) vs ~2us delivery. x is double-buffered
   across blocks. PSUM groups rotate over all 8 banks; each group closes
   after its 16-kt accumulation and is evacuated immediately (copies
   alternate ScalarE/VectorE into a 4-group staging tile; one store DMA
   per 4 groups, alternating sync/scalar rings).
 - A short warmup burst of junk matmuls (zeroed tile, PSUM bank 0) runs
   during the first DMA wait so the PE HAM clock-gate is already at full
   rate (2.4 GHz) when the real matmul stream begins.
"""

from contextlib import ExitStack

import numpy as np

import concourse.bacc as bacc
import concourse.bass as bass
import concourse.mybir as mybir
import concourse.tile as tile
from concourse.bass_utils import run_bass_kernel_spmd

N_CORES = 8
P = 128

FULL_N, FULL_OUT, FULL_IN = 16384, 2048, 2048


def build_nc(
    rows: int = FULL_N // N_CORES,
    in_dim: int = FULL_IN,
    out_dim: int = FULL_OUT,
    n_chunk: int = 512,
    m_block: int = 8,
    warmup_mms: int = 16,
    y_batch: int = 2,
):
    """Per-core Bass module: y[rows, out] = x @ (wt * m).

    DRAM layouts (all piece-contiguous, see _prep_host):
      wt [NT, 4, P, KQ*n_chunk] bf16   -- piece (nt, q) is [P, KQ*n_chunk]
      mk [NT, 4, P, KQ*n_chunk] int8
      x  [NB, 4, P, KQ*m_block*P] bf16 -- piece (b, q) is [P, KQ*mw]
      y  [rows, out_dim] f32 (row-major)
    """
    assert rows % P == 0 and in_dim % P == 0 and out_dim % n_chunk == 0
    KT = in_dim // P
    MT = rows // P
    NT = out_dim // n_chunk
    assert KT % 4 == 0 and MT % m_block == 0
    KQ = KT // 4
    NB = MT // m_block
    mw = m_block * P  # columns of x per block
    assert m_block % y_batch == 0

    bf16 = mybir.dt.bfloat16

    nc = bacc.Bacc("TRN2", target_bir_lowering=False, debug=False)
    x = nc.dram_tensor("x", [NB, 4, P, KQ * mw], bf16, kind="ExternalInput")
    wt = nc.dram_tensor("wt", [NT, 4, P, KQ * n_chunk], bf16, kind="ExternalInput")
    mk = nc.dram_tensor("mk", [NT, 4, P, KQ * n_chunk], mybir.dt.int8, kind="ExternalInput")
    y = nc.dram_tensor("y", [rows, out_dim], mybir.dt.float32, kind="ExternalOutput")

    with ExitStack() as ctx:
        tc = ctx.enter_context(tile.TileContext(nc))
        wm_pool = ctx.enter_context(tc.tile_pool(name="wm", bufs=1))
        mk_pool = ctx.enter_context(tc.tile_pool(name="mk", bufs=1))
        xt_pool = ctx.enter_context(tc.tile_pool(name="xt", bufs=1))
        yo_pool = ctx.enter_context(tc.tile_pool(name="yo", bufs=3))
        wu_pool = ctx.enter_context(tc.tile_pool(name="wu", bufs=1))
        pm_pool = ctx.enter_context(tc.tile_pool(name="pm", bufs=1, space="PSUM"))

        # Resident masked weight: wm_t[nt][q] of shape [P, KQ, n_chunk]
        wm_t = [
            [
                wm_pool.tile([P, KQ, n_chunk], bf16, tag=f"wm{nt}_{q}", name=f"wm{nt}_{q}")
                for q in range(4)
            ]
            for nt in range(NT)
        ]
        # Resident masks
        mk_t = [
            [
                mk_pool.tile(
                    [P, KQ, n_chunk], mybir.dt.int8, tag=f"mk{nt}_{q}", name=f"mk{nt}_{q}"
                )
                for q in range(4)
            ]
            for nt in range(NT)
        ]
        # x tiles: double-buffered per block parity: [set][q] -> [P, KQ, mw]
        xt_t = [
            [
                xt_pool.tile([P, KQ, mw], bf16, tag=f"xt{s}_{q}", name=f"xt{s}_{q}")
                for q in range(4)
            ]
            for s in range(2)
        ]

        # ---- PE warmup: junk matmuls into PSUM bank 0 ----
        # Values are irrelevant: the first real accumulation group on bank 0
        # opens with start=True, which clears has_written and overwrites.
        # Keeps the PE busy from the end of the engine-barrier preamble
        # (~6.5us) so HAM is at full clock when the real stream begins.
        if warmup_mms:
            wutile = wu_pool.tile([P, n_chunk], bf16, tag="wu", name="wu")
            nc.vector.memzero(wutile[:])
            pmw = pm_pool.tile([P, n_chunk], mybir.dt.float32, tag="pm0", name="pm_warm")
            for i in range(warmup_mms):
                nc.tensor.matmul(
                    pmw[:],
                    wutile[:, :P],
                    wutile[:],
                    start=(i == 0),
                    stop=(i == warmup_mms - 1),
                )

        def load_w_piece(nt, q):
            nc.sync.dma_start(
                out=wm_t[nt][q][:, :, :].rearrange("p a b -> p (a b)"),
                in_=wt[nt, q, :, :],
            )

        def mul_piece(nt, q):
            # per-k-slice muls: finer completion granularity, so the first
            # matmuls of a piece unblock ~2us earlier than one big mul
            for k in range(KQ):
                nc.vector.tensor_mul(
                    wm_t[nt][q][:, k, :], wm_t[nt][q][:, k, :], mk_t[nt][q][:, k, :]
                )

        def load_x_piece(b, q, eng):
            eng.dma_start(
                out=xt_t[b % 2][q][:, :, :].rearrange("p a b -> p (a b)"),
                in_=x[b, q, :, :],
            )

        def load_x_mslice(b, q, lo, hi, eng):
            xv = x[b, q, :, :].rearrange("p (a b) -> p a b", a=KQ)
            eng.dma_start(
                out=xt_t[b % 2][q][:, :, lo:hi],
                in_=xv[:, :, lo:hi],
            )

        # ---- prep: deadline-ordered across the two HWDGE rings ----
        # (gpsimd/SWDGE is avoided entirely: ~15us first-transfer latency and
        # it serializes issues on its own completion semaphores)
        # The nt0 window (~first 40us) is bandwidth-crunched: both HWDGE
        # rings carry deadline-critical bytes, split so W for nt1/nt3 rides
        # scalar (which goes quiet after the nt0 masks) instead of queuing
        # behind nt0's x quarters on sync.
        def load_mask(nt, q):
            nc.scalar.dma_start(out=mk_t[nt][q][:, :, :].rearrange("p a b -> p (a b)"),
                                in_=mk[nt, q, :, :])

        def load_w_piece_eng(nt, q, eng):
            eng.dma_start(
                out=wm_t[nt][q][:, :, :].rearrange("p a b -> p (a b)"),
                in_=wt[nt, q, :, :],
            )

        def load_w_khalf(nt, q, h, eng):
            # k-half of a W piece: more parallel streams early (HBM needs
            # outstanding requests to saturate); the per-k-slice muls gate on
            # subtile deps so the first slices unlock as soon as half A lands
            kh = KQ // 2
            eng.dma_start(
                out=wm_t[nt][q][:, h * kh : (h + 1) * kh, :].rearrange(
                    "p a b -> p (a b)"
                ),
                in_=wt[nt, q, :, h * kh * n_chunk : (h + 1) * kh * n_chunk],
            )

        # CONSTRUCTION ORDER MATTERS: a consumer must be issued after its
        # producers (dependencies only point backward in program order), so
        # each mask/W piece is constructed before the mul that reads it.
        # Ring plan -- the first-MM prefix is split across BOTH HWDGE rings
        # so its pieces transfer in parallel (a single early stream only
        # reaches ~150 GB/s; HBM needs several outstanding streams):
        #   sync:   xq0 (2 halves), W(0,2..3), xq3, W nt1-3, block-1 x
        #   scalar: W(0,0), m(0,0), W(0,1), m(0,1..3), masks nt1-3, evac/y
        #   gpsimd: xq1, xq2 only (SWDGE startup ~15us; deadlines 19/26us)
        # Block-1 x is last on sync: same-ring FIFO is the only real
        # throttle that keeps it out of the ramp crunch.
        load_w_piece_eng(0, 0, nc.scalar)
        load_mask(0, 0)
        mul_piece(0, 0)
        # xq0 streams in m-quarters: the first matmuls start as soon as the
        # first 2 m-tiles land instead of waiting for the full piece
        for i in range(4):
            load_x_mslice(0, 0, i * mw // 4, (i + 1) * mw // 4, nc.sync)
        load_w_piece_eng(0, 1, nc.scalar)
        load_mask(0, 1)
        mul_piece(0, 1)
        for q in (2, 3):
            load_w_piece(0, q)
            load_mask(0, q)
            mul_piece(0, q)
        load_x_piece(0, 1, nc.gpsimd)
        load_x_piece(0, 2, nc.gpsimd)
        load_x_piece(0, 3, nc.sync)
        # nt1's first W pieces ride gpsimd behind xq2: its big-descriptor
        # queue holds a high packet share, landing them ~20us before the
        # sync ring (busy with xq3 + the rest of W) could
        load_w_piece_eng(1, 0, nc.gpsimd)
        load_w_piece_eng(1, 1, nc.gpsimd)
        for q in range(4):
            load_mask(1, q)
        load_w_piece(1, 2)
        load_w_piece(1, 3)
        for nt in (2, 3):
            for q in range(4):
                load_w_piece(nt, q)
        for nt in (2, 3):
            for q in range(4):
                load_mask(nt, q)
        if NB > 1:
            for q in range(4):
                load_x_piece(1, q, nc.sync)
        # NOTE: muls for nt>=1 are issued inside the main loop (interleaved
        # with nt-1's matmuls) so VectorE's strict-FIFO queue never parks a
        # late mask-multiply in front of PSUM evacuation copies.

        # ---- main: blocks of m_block m-tiles; nt-outer inside a block ----
        evac_i = 0
        for b in range(NB):
            xts = xt_t[b % 2]
            for nt in range(NT):
                # 8 rotating PSUM banks: group (nt, mb) lives on bank mb
                pms = {
                    mb: pm_pool.tile(
                        [P, n_chunk],
                        mybir.dt.float32,
                        tag=f"pm{(nt * m_block + mb) % 8}",
                        name=f"pm{(nt * m_block + mb) % 8}",
                    )
                    for mb in range(m_block)
                }
                if b == 0 and nt + 1 < NT:
                    # issue next nt's mask-multiplies now: they complete
                    # (~11us) well inside this nt's ~27us of matmuls, and
                    # VectorE's FIFO stays clear for evacuation copies
                    for q in range(4):
                        mul_piece(nt + 1, q)
                yo = None
                for q in range(4):
                    for mb in range(m_block):
                        for k in range(KQ):
                            kt = q * KQ + k
                            nc.tensor.matmul(
                                pms[mb][:],
                                xts[q][:, k, bass.ts(mb, P)],
                                wm_t[nt][q][:, k, :],
                                start=(kt == 0),
                                stop=(kt == KT - 1),
                            )
                        if q == 3:
                            # evacuate on close: copy into a batch-wide
                            # staging tile, one store DMA per batch.
                            # Block 0: copies are Scalar-only -- VectorE's
                            # strict FIFO still holds pending mask-muls, and a
                            # copy parked behind them would delay PSUM bank
                            # frees. Block 1+: alternate Scalar/VectorE.
                            yb = y_batch
                            j = mb % y_batch
                            if j == 0:
                                yo = yo_pool.tile(
                                    [P, yb, n_chunk], mybir.dt.float32, tag="yo"
                                )
                            if b == 0 or evac_i % 2 == 0:
                                nc.scalar.copy(yo[:, j, :], pms[mb][:])
                            else:
                                nc.vector.tensor_copy(yo[:, j, :], pms[mb][:])
                            evac_i += 1
                            if j == yb - 1:
                                mt0 = b * m_block + mb - (yb - 1)
                                dst = y[
                                    mt0 * P : (mt0 + yb) * P, bass.ts(nt, n_chunk)
                                ].rearrange("(j p) n -> p j n", p=P)
                                # all stores on scalar: HWDGE pipelines issues
                                # without waiting on completions (SWDGE does
                                # not), and sync stays clear for inputs
                                nc.scalar.dma_start(out=dst, in_=yo[:])

    nc.compile()
    return nc


def _prep_host(input_, weight, mask, n_chunk=512, m_block=8):
    import ml_dtypes

    in_dim, out_dim = weight.shape[1], weight.shape[0]
    nt = out_dim // n_chunk
    kt = in_dim // P
    kq = kt // 4
    rows = input_.shape[0] // N_CORES
    mw = m_block * P
    nb = rows // mw

    # W.T [IN, OUT] -> pieces [NT, 4, P, KQ*n_chunk]:
    # piece (t, q)[p, kl*n_chunk + n] = W.T[(4q+kl)*128 + p, t*n_chunk + n]
    wtT = weight.T.reshape(4, kq, P, nt, n_chunk)  # [q, kl, p, t, n]
    wtp = np.ascontiguousarray(wtT.transpose(3, 0, 2, 1, 4)).reshape(
        nt, 4, P, kq * n_chunk
    ).astype(ml_dtypes.bfloat16)
    mkT = mask.T.reshape(4, kq, P, nt, n_chunk)
    mkp = np.ascontiguousarray(mkT.transpose(3, 0, 2, 1, 4)).reshape(
        nt, 4, P, kq * n_chunk
    ).astype(np.int8)

    xbf = input_.astype(ml_dtypes.bfloat16)
    in_maps = []
    for c in range(N_CORES):
        xc = xbf[c * rows : (c + 1) * rows].T  # [IN, rows]
        # pieces [NB, 4, P, KQ*mw]: (b, q)[p, kl*mw + m] = xc[(4q+kl)*128+p, b*mw+m]
        xr = xc.reshape(4, kq, P, nb, mw)  # [q, kl, p, b, m]
        xp = np.ascontiguousarray(xr.transpose(3, 0, 2, 1, 4)).reshape(
            nb, 4, P, kq * mw
        )
        in_maps.append({"x": xp, "wt": wtp, "mk": mkp})
    return in_maps


_CACHE = {}


def _run(input_, weight, mask, trace=False, **build_kw):
    rows_total, in_dim = input_.shape
    out_dim = weight.shape[0]
    key = (rows_total, in_dim, out_dim, tuple(sorted(build_kw.items())))
    if key not in _CACHE:
        _CACHE[key] = build_nc(
            rows=rows_total // N_CORES, in_dim=in_dim, out_dim=out_dim, **build_kw
        )
    nc = _CACHE[key]
    in_maps = _prep_host(
        input_,
        weight,
        mask,
        build_kw.get("n_chunk", 512),
        build_kw.get("m_block", 8),
    )
    res = run_bass_kernel_spmd(nc, in_maps, core_ids=list(range(N_CORES)), trace=trace)
    out = np.concatenate([res.results[c]["y"] for c in range(N_CORES)], axis=0)
    return out, res


def kernel(input_, weight, mask):
    input_ = np.asarray(input_, dtype=np.float32)
    weight = np.asarray(weight, dtype=np.float32)
    mask = np.asarray(mask)
    out, _ = _run(input_, weight, mask, trace=False)
    return out
